# revision 21
# baseline (speedup 1.0000x reference)
"""AxisAttention TRN2 Bass kernel (fp8 DoubleRow + engine-balanced schedule).

Full-input contract: kernel(**inputs) takes the unsharded numpy inputs and
returns the full [4, 2048, 512] float32 output.

Sharding: data-parallel over (batch, query-half) -> 8 NeuronCores. Each core
computes attention for 1024 queries of one batch against that batch's full
2048 keys. Params are replicated; K/V projections are recomputed by the two
cores sharing a batch (a pair-AllGather K-dedup was measured ~38us for the
1MB exchange under this runtime — far slower than recomputing; KDEDUP=False).

Math per core (n=1024 queries, m=2048 keys, d=a=c=512), mixed precision:
  qT[a,n]  = sum_d WqS[d,a] * xqT[d,n]        fp16 (WqS = Wq*sqrt(512))
  kT[a,m]  = sum_d Wk[d,a] * xkvT[d,m]        fp16
  v'[m,c]  = sum_d xkv8[d,m] * (32*Wv)[d,c]   fp8 DoubleRow (2 rows/cyc)
  S[n,m]   = sum_a qT[a,n] * kT[a,m]          fp16, PSUM f32
  P8       = fp8(exp(S - rowmax)) via ACT directly; rowsum f32 accum
  PT       = xbar transpose of P8 as packed uint16 pairs; the pair byte
             index becomes the DoubleRow i-index (v' is computed in a
             matching host-permuted key order)
  OT8[c,n] = fp8((1/16) * sum_m v'[m,c] * PT[m,n])   fp8 DoubleRow
  Y[n,dq]  = sum_c OT8[c,n] * (32*Wo)[c,dq]          fp8 DoubleRow
  out[n,:] = Y/(64*rowsum) + query16[n,:]  (fp16 store; f32 cast on host)

Schedule (PE program order): warmup (ramps the PE clock while inputs load),
Q-proj, K-proj, S0..7, V-proj (covers the softmax drain), PV ck0, Y t0-3,
PV ck1, Y t4-7. DMA issue is spread over gpsimd/sync/scalar sequencers
(DIRECT2D costs ~0.7-2.2us serial per issue); engine balance: DVE does qT
casts + softmax stats + finals, ACT does exp + kT/vv8/OT8 copies, GpSimd
issues loads/stores. Measured: ~104us HW exec, rel err 1.085e-2 (vs
129.4us fp16 baseline).
"""

import numpy as np
import ml_dtypes

import concourse.bass as bass
import concourse.mybir as mybir
import concourse.tile as tile
from concourse import bacc
from concourse.bass_utils import run_bass_kernel_spmd

F8 = mybir.dt.float8e4
F16 = mybir.dt.float16
F32 = mybir.dt.float32
U16 = mybir.dt.uint16
AX = mybir.AxisListType
ALU = mybir.AluOpType
ACTF = mybir.ActivationFunctionType
DR = mybir.MatmulPerfMode.DoubleRow
NPF8 = ml_dtypes.float8_e4m3

B, N, D = 4, 2048, 512
N_CORES = 8
NQ = N // 2          # 1024 queries per core
M = N                # 2048 keys per core
P = 128              # partitions
SCALE = float(np.sqrt(float(D)))
WSC = 32.0           # fp8 weight prescale (Wv, Wo)
PSC = 16.0           # OT8 copy downscale

ND = D // P          # 4 contraction chunks of 128
NNT = NQ // P        # 8 query tiles of 128
NMT = M // P         # 16 key tiles of 128
NMC = M // 512       # 4 key chunks of 512
NCH = NQ // 512      # 2 query chunks of 512
NMJ = M // 256       # 8 key pair-blocks of 256

KDEDUP = False        # split k-projection across the core pair + AllGather


def _sl(i, w=P):
    return slice(i * w, (i + 1) * w)


def _build(with_bqk: bool, with_bv: bool, with_bo: bool):
    nc = bacc.Bacc("TRN2", target_bir_lowering=False, debug=False,
                   num_devices=N_CORES)

    nkc = 2 if KDEDUP else NMC   # xkvT chunks loaded/projected locally
    wq = nc.dram_tensor("wq16", [P, ND, D], F16, kind="ExternalInput").ap()
    wk = nc.dram_tensor("wk16", [P, ND, D], F16, kind="ExternalInput").ap()
    wv8 = nc.dram_tensor("wv8", [P, ND, D], F8, kind="ExternalInput").ap()
    wo8 = nc.dram_tensor("wo8", [P, ND, D], F8, kind="ExternalInput").ap()
    xqT = nc.dram_tensor("xqT16", [NCH, P, ND, 512], F16,
                         kind="ExternalInput").ap()
    xkvT = nc.dram_tensor("xkvT16", [nkc, P, ND, 512], F16,
                          kind="ExternalInput").ap()
    xkv8 = nc.dram_tensor("xkv8", [2, P, ND, M // 2], F8,
                          kind="ExternalInput").ap()
    xqn = nc.dram_tensor("xq16n", [2, P, NNT // 2, D], F16,
                         kind="ExternalInput").ap()
    bq = nc.dram_tensor("bq", [D], F32, kind="ExternalInput").ap()
    bk = nc.dram_tensor("bk", [D], F32, kind="ExternalInput").ap()
    bv8 = nc.dram_tensor("bv8", [1, D], F8, kind="ExternalInput").ap()
    bo32 = nc.dram_tensor("bo32", [1, D], F32, kind="ExternalInput").ap()
    out = nc.dram_tensor("out", [NNT, P, D], F16, kind="ExternalOutput").ap()
    if KDEDUP:
        kloc_d = nc.dram_tensor("kloc_d", [1, P, ND, M // 2], F16).ap()
        kgath_d = nc.dram_tensor("kgath_d", [2, P, ND, M // 2], F16).ap()

    with tile.TileContext(nc) as tc:
        with tc.tile_pool(name="pers", bufs=1) as pers:
            # ---- input loads: issue spread over gpsimd/sync/scalar, in
            # consumption order per engine; wq/xqT0 split for earliest start.
            WQ = pers.tile([P, ND, D], F16, name="wq", tag="wq")
            WK = pers.tile([P, ND, D], F16, name="wk", tag="wk")
            WV8 = pers.tile([P, ND, D], F8, name="wv8", tag="wv8")
            WO8 = pers.tile([P, ND, D], F8, name="wo8", tag="wo8")
            XQT = pers.tile([P, ND, NQ], F16, name="xqt", tag="xqt")
            XKVT = pers.tile([P, ND, 512 * nkc], F16, name="xkvt", tag="xkvt")
            XKV8 = pers.tile([P, ND, M], F8, name="xkv8", tag="xkv8")
            XQN = pers.tile([P, NNT, D], F16, name="xqn", tag="xqn")

            nc.gpsimd.dma_start(out=WQ[:, 0:2, :], in_=wq[:, 0:2, :])
            nc.gpsimd.dma_start(out=WQ[:, 2:4, :], in_=wq[:, 2:4, :])
            nc.sync.dma_start(out=XQT[:, 0:2, 0:512], in_=xqT[0, :, 0:2, :])
            nc.sync.dma_start(out=XQT[:, 2:4, 0:512], in_=xqT[0, :, 2:4, :])
            nc.sync.dma_start(out=XQT[:, :, 512:1024], in_=xqT[1])
            nc.gpsimd.dma_start(out=WK[:], in_=wk[:])
            for c in range(nkc):
                eng = nc.scalar if c % 2 == 0 else nc.sync
                eng.dma_start(out=XKVT[:, :, _sl(c, 512)], in_=xkvT[c])
            for g in range(2):
                eng = nc.scalar if g % 2 == 0 else nc.sync
                eng.dma_start(out=XKV8[:, :, _sl(g, M // 2)], in_=xkv8[g])
            nc.gpsimd.dma_start(out=WV8[:], in_=wv8[:])
            nc.gpsimd.dma_start(out=WO8[:], in_=wo8[:])
            for h in range(2):
                nc.gpsimd.dma_start(
                    out=XQN[:, h * (NNT // 2):(h + 1) * (NNT // 2), :],
                    in_=xqn[h])
            if with_bqk:
                BQ = [pers.tile([P, 1], F32, name=f"bq{i}", tag=f"bq{i}")
                      for i in range(ND)]
                BK = [pers.tile([P, 1], F32, name=f"bk{i}", tag=f"bk{i}")
                      for i in range(ND)]
                for i in range(ND):
                    nc.gpsimd.dma_start(out=BQ[i][:],
                                        in_=bq[_sl(i)].rearrange("(a b) -> a b", b=1))
                    nc.gpsimd.dma_start(out=BK[i][:],
                                        in_=bk[_sl(i)].rearrange("(a b) -> a b", b=1))
            if with_bv:
                BV8 = pers.tile([1, D], F8, name="bv8", tag="bv8")
                ONES8 = pers.tile([1, P], F8, name="ones8", tag="ones8")
                nc.gpsimd.dma_start(out=BV8[:], in_=bv8[:])
                nc.gpsimd.memset(ONES8[:], 1.0)
            if with_bo:
                BO = pers.tile([1, D], F32, name="bo", tag="bo")
                BOB = pers.tile([P, D], F32, name="bob", tag="bob")
                nc.gpsimd.dma_start(out=BO[:], in_=bo32[:])
                nc.gpsimd.partition_broadcast(BOB[:], BO[:])

            # ---- PE warmup: ramp the tensor-engine clock while inputs load
            WARM = pers.tile([P, 640], F16, name="warm", tag="warm")
            nc.vector.memset(WARM[:], 0.5)
            with tc.tile_pool(name="wps", bufs=1, space="PSUM") as wpool:
                wps = wpool.tile([P, 512], F32, name="w", tag="w")
                NWARM = 13
                for i in range(NWARM):
                    nc.tensor.matmul(wps[:], WARM[:, 0:128], WARM[:, 128:640],
                                     start=(i == 0), stop=(i == NWARM - 1))

            # ---- projections ------------------------------------------------
            qT = pers.tile([P, ND, NQ], F16, name="qT", tag="qT")
            kT = pers.tile([P, ND, M], F16, name="kT", tag="kT")
            if KDEDUP:
                kloc = pers.tile([P, ND, M // 2], F16, name="kloc",
                                 tag="kloc")
            vv8 = pers.tile([P, NMT, D], F8, name="vv8", tag="vv8")

            with tc.tile_pool(name="pps", bufs=6, space="PSUM") as pps:
                # q projection (fp16)
                for a in range(ND):
                    pss = [pps.tile([P, 512], F32, name="projps", tag="projps")
                           for _ in range(NCH)]
                    for d in range(ND):
                        for c in range(NCH):
                            nc.tensor.matmul(pss[c][:], WQ[:, d, _sl(a)],
                                             XQT[:, d, _sl(c, 512)],
                                             start=(d == 0), stop=(d == ND - 1))
                    for c in range(NCH):
                        if with_bqk:
                            nc.vector.tensor_scalar_add(
                                qT[:, a, _sl(c, 512)], pss[c][:], BQ[a][:])
                        else:
                            nc.vector.tensor_copy(qT[:, a, _sl(c, 512)],
                                                  pss[c][:])
                # k projection over the local chunks; m-chunk outer so each
                # xkvT chunk is consumed as it lands; copies alternate
                # scalar/vector to drain 2x.
                ktgt = kloc if KDEDUP else kT
                for c in range(nkc):
                    pss = [pps.tile([P, 512], F32, name="projps", tag="projps")
                           for _ in range(ND)]
                    for a in range(ND):
                        for d in range(ND):
                            nc.tensor.matmul(pss[a][:], WK[:, d, _sl(a)],
                                             XKVT[:, d, _sl(c, 512)],
                                             start=(d == 0), stop=(d == ND - 1))
                    for a in range(ND):
                        if with_bqk:
                            if a % 2 == 0:
                                nc.scalar.add(ktgt[:, a, _sl(c, 512)],
                                              pss[a][:], add=BK[a][:])
                            else:
                                nc.vector.tensor_scalar_add(
                                    ktgt[:, a, _sl(c, 512)], pss[a][:],
                                    BK[a][:])
                        elif a % 2 == 0:
                            nc.scalar.copy(ktgt[:, a, _sl(c, 512)], pss[a][:])
                        else:
                            nc.vector.tensor_copy(ktgt[:, a, _sl(c, 512)],
                                                  pss[a][:])
                if KDEDUP:
                    # pair-exchange: kloc -> DRAM -> AllGather -> full kT.
                    # Issued on gpsimd; the v-projection below overlaps the
                    # collective latency on the PE.
                    nc.gpsimd.dma_start(out=kloc_d[0], in_=kloc[:])
                    nc.gpsimd.collective_compute(
                        "AllGather", ALU.bypass,
                        replica_groups=[[0, 1], [2, 3], [4, 5], [6, 7]],
                        ins=[kloc_d[:]], outs=[kgath_d[:]])
                    for g in range(2):
                        for half in range(2):
                            eng = nc.sync if half == 0 else nc.scalar
                            eng.dma_start(
                                out=kT[:, 2 * half:2 * half + 2,
                                       _sl(g, M // 2)],
                                in_=kgath_d[g, :, 2 * half:2 * half + 2, :])

            # ---- scores + softmax + PV + out-proj ---------------------------
            # PT16[ck][p, mj, n] = packed fp8 pair
            #   (P8[512ck+n, 256mj+2p], P8[512ck+n, 256mj+2p+1])
            PT16 = [pers.tile([P, NMJ, 512], U16, name=f"PT16{ck}",
                              tag=f"PT16{ck}") for ck in range(NCH)]
            PT8 = [PT16[ck][:].bitcast(F8).rearrange("p b (n l) -> p b l n",
                                                     l=2)
                   for ck in range(NCH)]
            OT8 = pers.tile([P, ND, NQ], F8, name="OT8", tag="OT8")
            rf = [None] * NNT  # per-tile final scale (1/(64*rowsum))

            def softmax_tile(t, spool, p8pool, stat):
                # two [P, 1024] psum half-tiles (2 banks each) so the ovy
                # pool can co-reside and PV starts right after S7.
                pt8 = p8pool.tile([P, M], F8, name="P8", tag="P8")
                halves = []
                nmh = []
                for h in range(2):
                    sps = spool.tile([P, M // 2], F32, name=f"S{h}", tag="S")
                    for a in range(ND):
                        for c in range(2):
                            mc = h * 2 + c
                            nc.tensor.matmul(sps[:, _sl(c, 512)],
                                             qT[:, a, _sl(t)],
                                             kT[:, a, _sl(mc, 512)],
                                             start=(a == 0), stop=(a == ND - 1))
                    nm = stat.tile([P, 1], F32, name=f"nm{h}", tag=f"nm{h}")
                    nc.vector.tensor_reduce(nm[:], sps[:], axis=AX.X,
                                            op=ALU.max, negate=True)
                    halves.append(sps)
                    nmh.append(nm)
                negmax = stat.tile([P, 1], F32, name="negmax", tag="negmax")
                nc.vector.tensor_tensor(negmax[:], nmh[0][:], nmh[1][:],
                                        op=ALU.min)
                # exp writes raw P8 (fp8) directly; rowsum accumulated f32.
                # Normalization is folded into the final per-tile scale.
                rsh = []
                for h in range(2):
                    rs = stat.tile([P, 1], F32, name=f"rs{h}", tag=f"rs{h}")
                    nc.scalar.activation(pt8[:, _sl(h, M // 2)], halves[h][:],
                                         ACTF.Exp, bias=negmax[:], scale=1.0,
                                         accum_out=rs[:])
                    rsh.append(rs)
                rowsum = stat.tile([P, 1], F32, name="rowsum", tag="rowsum")
                nc.vector.tensor_tensor(rowsum[:], rsh[0][:], rsh[1][:],
                                        op=ALU.add)
                recip = stat.tile([P, 1], F32, name="recip", tag="recip")
                nc.vector.reciprocal(recip[:], rowsum[:])
                rf[t] = stat.tile([P, 1], F32, name="rf", tag="rf")
                nc.vector.tensor_scalar_mul(rf[t][:], recip[:], 1.0 / 64.0)
                # packed-pair transpose, split per query-chunk tile
                ck, tt = divmod(t, 4)
                nc.sync.dma_start(out=PT16[ck][:, :, _sl(tt)],
                                  in_=pt8[:].bitcast(U16), transpose=True)

            with tc.tile_pool(name="p8pool", bufs=2) as p8pool, \
                 tc.tile_pool(name="stat", bufs=8) as stat, \
                 tc.tile_pool(name="fin", bufs=3) as fin, \
                 tc.tile_pool(name="ovy", bufs=2, space="PSUM") as ovy:
                with tc.tile_pool(name="spool", bufs=3, space="PSUM") as spool:
                    for t in range(NNT):
                        softmax_tile(t, spool, p8pool, stat)

                # v projection (fp8 DoubleRow) here: overlaps the softmax
                # pipeline drain (exp+transpose of the last tiles).
                for vt in range(NMT):
                    ps = ovy.tile([P, 512], F32, name="ot", tag="ot")
                    for g in range(2):
                        last = (g == 1) and not with_bv
                        nc.tensor.matmul(
                            ps[:], XKV8[:, 2 * g:2 * g + 2, _sl(vt)],
                            WV8[:, 2 * g:2 * g + 2, :],
                            start=(g == 0), stop=last, perf_mode=DR)
                    if with_bv:
                        nc.tensor.matmul(ps[:], ONES8[:], BV8[:],
                                         start=False, stop=True)
                    if vt % 2 == 0:
                        nc.scalar.copy(vv8[:, vt, :], ps[:])
                    else:
                        nc.vector.tensor_copy(vv8[:, vt, :], ps[:])

                with tc.tile_pool(name="ypool", bufs=2, space="PSUM") as ypool:
                    def y_tile(t):
                        ps = ypool.tile([P, D], F32, name="y", tag="y")
                        for h in range(2):
                            nc.tensor.matmul(ps[:],
                                             OT8[:, 2 * h:2 * h + 2, _sl(t)],
                                             WO8[:, 2 * h:2 * h + 2, :],
                                             start=(h == 0), stop=(h == 1),
                                             perf_mode=DR)
                        osb = fin.tile([P, D], F16, name="osb", tag="osb")
                        nc.vector.scalar_tensor_tensor(
                            out=osb[:], in0=ps[:], scalar=rf[t][:],
                            in1=XQN[:, t, :], op0=ALU.mult, op1=ALU.add)
                        if with_bo:
                            nc.vector.tensor_add(osb[:], osb[:], BOB[:])
                        nc.gpsimd.dma_start(out=out[t], in_=osb[:])

                    # PV chunk ck covers query cols 512ck..512ck+512 (tiles
                    # t=4ck..4ck+3); its Y tiles run right after so output
                    # stores spread across the PV phase.
                    for ck in range(NCH):
                        for ct in range(ND):
                            ps = ovy.tile([P, 512], F32, name="ot", tag="ot")
                            for mj in range(NMJ):
                                nc.tensor.matmul(
                                    ps[:], vv8[:, 2 * mj:2 * mj + 2, _sl(ct)],
                                    PT8[ck][:, mj, :, :],
                                    start=(mj == 0), stop=(mj == NMJ - 1),
                                    perf_mode=DR)
                            nc.scalar.activation(OT8[:, ct, _sl(ck, 512)],
                                                 ps[:], ACTF.Copy,
                                                 scale=1.0 / PSC)
                        for t in range(4 * ck, 4 * ck + 4):
                            y_tile(t)

    nc.compile()
    return nc


_BUILD_CACHE = {}


def _get_nc(with_bqk: bool, with_bv: bool, with_bo: bool):
    key = (with_bqk, with_bv, with_bo)
    if key not in _BUILD_CACHE:
        _BUILD_CACHE[key] = _build(with_bqk, with_bv, with_bo)
    return _BUILD_CACHE[key]


# key permutation for the packed-pair transpose: v'-block vt=2*mj+l holds
# keys m = 256*mj + 2*q + l (q = partition)
_J = np.arange(M)
_MIDX = 256 * (_J >> 8) + 2 * (_J & 127) + ((_J >> 7) & 1)


def kernel(query, key_value, Wq, bq, Wk, bk, Wv, bv, Wo, bo, _timing=None):
    query = np.asarray(query, dtype=np.float32)
    key_value = np.asarray(key_value, dtype=np.float32)
    Wq = np.asarray(Wq, dtype=np.float32)
    Wk = np.asarray(Wk, dtype=np.float32)
    Wv = np.asarray(Wv, dtype=np.float32)
    Wo = np.asarray(Wo, dtype=np.float32)
    bq = np.asarray(bq, dtype=np.float32)
    bk = np.asarray(bk, dtype=np.float32)
    bv = np.asarray(bv, dtype=np.float32)
    bo = np.asarray(bo, dtype=np.float32)

    with_bqk = bool(np.any(bq)) or bool(np.any(bk))
    with_bv = bool(np.any(bv))
    with_bo = bool(np.any(bo))
    nc = _get_nc(with_bqk, with_bv, with_bo)

    def packw(w, dtype, scale=1.0):
        # [512, 512] -> [128, 4, 512] with [p, d, :] = w[d*128+p, :]
        return np.ascontiguousarray(
            (w * scale).reshape(ND, P, D).transpose(1, 0, 2).astype(dtype))

    wq16 = packw(Wq, np.float16, SCALE)
    wk16 = packw(Wk, np.float16)
    wv8 = packw(Wv, NPF8, WSC)
    wo8 = packw(Wo, NPF8, WSC)
    bqs = (bq * SCALE).astype(np.float32)
    bk32 = bk.astype(np.float32)
    bv8_h = (bv * WSC).astype(NPF8).reshape(1, D)
    bo32 = bo.astype(np.float32).reshape(1, D)

    q16 = query.astype(np.float16)
    kv16 = key_value.astype(np.float16)
    kv8 = key_value.astype(NPF8)

    in_maps = []
    for core in range(N_CORES):
        b, h = divmod(core, 2)
        qh = q16[b, h * NQ:(h + 1) * NQ]                   # [1024, 512] f16
        # xqT16 [2, 128, 4, 512]: [c, p, d, nn] = qh[c*512+nn, d*128+p]
        xqT = qh.T.reshape(ND, P, NCH, 512).transpose(2, 1, 0, 3)
        # xkvT16 [4, 128, 4, 512]: [j, p, d, mm] = kv[512j+mm, d*128+p]
        xkvT = kv16[b].T.reshape(ND, P, NMC, 512).transpose(2, 1, 0, 3)
        if KDEDUP:
            xkvT = xkvT[2 * h:2 * h + 2]                   # local key half
        # xkv8 [2, 128, 4, 1024]: permuted key order
        kv8p = kv8[b][_MIDX]                                # [2048, 512] f8
        xkv8_h = kv8p.T.reshape(ND, P, 2, M // 2).transpose(2, 1, 0, 3)
        # xq16n [2, 128, 4, 512]: [h2, p, tt, j] = qh[(4*h2+tt)*128+p, j]
        xqn = qh.reshape(2, NNT // 2, P, D).transpose(0, 2, 1, 3)
        im = {
            "xqT16": np.ascontiguousarray(xqT),
            "xkvT16": np.ascontiguousarray(xkvT),
            "xkv8": np.ascontiguousarray(xkv8_h),
            "xq16n": np.ascontiguousarray(xqn),
            "wq16": wq16, "wk16": wk16, "wv8": wv8, "wo8": wo8,
            "bq": bqs, "bk": bk32, "bv8": bv8_h, "bo32": bo32,
        }
        in_maps.append(im)

    res = run_bass_kernel_spmd(nc, in_maps, list(range(N_CORES)),
                               **(_timing or {}))
    out = np.empty((B, N, D), dtype=np.float32)
    for core in range(N_CORES):
        b, h = divmod(core, 2)
        out[b, h * NQ:(h + 1) * NQ] = (
            res.results[core]["out"].reshape(NQ, D).astype(np.float32))
    if _timing is not None:
        return out, res
    return out


# revision 22
# speedup vs baseline: 1.2684x; 1.2684x over previous
"""AxisAttention TRN2 Bass kernel (fp8 DoubleRow + engine-balanced schedule).

Full-input contract: kernel(**inputs) takes the unsharded numpy inputs and
returns the full [4, 2048, 512] float32 output.

Sharding: data-parallel over (batch, query-half) -> 8 NeuronCores. Each core
computes attention for 1024 queries of one batch against that batch's full
2048 keys. Params are replicated; K/V projections are recomputed by the two
cores sharing a batch (a pair-AllGather K-dedup was measured ~38us for the
1MB exchange under this runtime — far slower than recomputing; KDEDUP=False).

Math per core (n=1024 queries, m=2048 keys, d=a=c=512), mixed precision:
  qT[a,n]  = sum_d WqS[d,a] * xqT[d,n]        fp16 (WqS = Wq*sqrt(512))
  kT[a,m]  = sum_d Wk[d,a] * xkvT[d,m]        fp16
  v'[m,c]  = sum_d xkv8[d,m] * (32*Wv)[d,c]   fp8 DoubleRow (2 rows/cyc)
  S[n,m]   = sum_a qT[a,n] * kT[a,m]          fp16, PSUM f32
  P8       = fp8(exp(S - rowmax)) via ACT directly; rowsum f32 accum
  PT       = xbar transpose of P8 as packed uint16 pairs; the pair byte
             index becomes the DoubleRow i-index (v' is computed in a
             matching host-permuted key order)
  OT8[c,n] = fp8((1/16) * sum_m v'[m,c] * PT[m,n])   fp8 DoubleRow
  Y[n,dq]  = sum_c OT8[c,n] * (32*Wo)[c,dq]          fp8 DoubleRow
  out[n,:] = Y/(64*rowsum) + query16[n,:]  (fp16 store; f32 cast on host)

Schedule (PE program order): warmup (ramps the PE clock while inputs load),
Q-proj, K-proj, S0..7, V-proj (covers the softmax drain), PV ck0, Y t0-3,
PV ck1, Y t4-7. DMA issue is spread over gpsimd/sync/scalar sequencers
(DIRECT2D costs ~0.7-2.2us serial per issue); engine balance: DVE does qT
casts + softmax stats + finals, ACT does exp + kT/vv8/OT8 copies, GpSimd
issues loads/stores. Measured: ~104us HW exec, rel err 1.085e-2 (vs
129.4us fp16 baseline).
"""

import numpy as np
import ml_dtypes

import concourse.bass as bass
import concourse.mybir as mybir
import concourse.tile as tile
from concourse import bacc
from concourse.bass_utils import run_bass_kernel_spmd

F8 = mybir.dt.float8e4
F16 = mybir.dt.float16
F32 = mybir.dt.float32
U16 = mybir.dt.uint16
AX = mybir.AxisListType
ALU = mybir.AluOpType
ACTF = mybir.ActivationFunctionType
DR = mybir.MatmulPerfMode.DoubleRow
NPF8 = ml_dtypes.float8_e4m3

B, N, D = 4, 2048, 512
N_CORES = 8
NQ = N // 2          # 1024 queries per core
M = N                # 2048 keys per core
P = 128              # partitions
SCALE = float(np.sqrt(float(D)))
WSC = 32.0           # fp8 weight prescale (Wv, Wo)
PSC = 16.0           # OT8 copy downscale

ND = D // P          # 4 contraction chunks of 128
NNT = NQ // P        # 8 query tiles of 128
NMT = M // P         # 16 key tiles of 128
NMC = M // 512       # 4 key chunks of 512
NCH = NQ // 512      # 2 query chunks of 512
NMJ = M // 256       # 8 key pair-blocks of 256

KDEDUP = False        # split k-projection across the core pair + AllGather


def _sl(i, w=P):
    return slice(i * w, (i + 1) * w)


def _build(with_bqk: bool, with_bv: bool, with_bo: bool):
    nc = bacc.Bacc("TRN2", target_bir_lowering=False, debug=False,
                   num_devices=N_CORES)

    nkc = 2 if KDEDUP else NMC   # xkvT chunks loaded/projected locally
    wq = nc.dram_tensor("wq16", [P, ND, D], F16, kind="ExternalInput").ap()
    wk = nc.dram_tensor("wk16", [P, ND, D], F16, kind="ExternalInput").ap()
    wv8 = nc.dram_tensor("wv8", [P, ND, D], F8, kind="ExternalInput").ap()
    wo8 = nc.dram_tensor("wo8", [P, ND, D], F8, kind="ExternalInput").ap()
    xqT = nc.dram_tensor("xqT16", [NCH, P, ND, 512], F16,
                         kind="ExternalInput").ap()
    xkvT = nc.dram_tensor("xkvT16", [nkc, P, ND, 512], F16,
                          kind="ExternalInput").ap()
    xkv8 = nc.dram_tensor("xkv8", [2, P, ND, M // 2], F8,
                          kind="ExternalInput").ap()
    xqn = nc.dram_tensor("xq16n", [2, P, NNT // 2, D], F16,
                         kind="ExternalInput").ap()
    bq = nc.dram_tensor("bq", [D], F32, kind="ExternalInput").ap()
    bk = nc.dram_tensor("bk", [D], F32, kind="ExternalInput").ap()
    bv8 = nc.dram_tensor("bv8", [1, D], F8, kind="ExternalInput").ap()
    bo32 = nc.dram_tensor("bo32", [1, D], F32, kind="ExternalInput").ap()
    out = nc.dram_tensor("out", [NNT, P, D], F16, kind="ExternalOutput").ap()
    if KDEDUP:
        kloc_d = nc.dram_tensor("kloc_d", [1, P, ND, M // 2], F16).ap()
        kgath_d = nc.dram_tensor("kgath_d", [2, P, ND, M // 2], F16).ap()

    with tile.TileContext(nc) as tc:
        with tc.tile_pool(name="pers", bufs=1) as pers:
            # ---- input loads: issue spread over gpsimd/sync/scalar, in
            # consumption order per engine; wq/xqT0 split for earliest start.
            WQ = pers.tile([P, ND, D], F16, name="wq", tag="wq")
            WK = pers.tile([P, ND, D], F16, name="wk", tag="wk")
            WV8 = pers.tile([P, ND, D], F8, name="wv8", tag="wv8")
            WO8 = pers.tile([P, ND, D], F8, name="wo8", tag="wo8")
            XQT = pers.tile([P, ND, NQ], F16, name="xqt", tag="xqt")
            XKVT = pers.tile([P, ND, 512 * nkc], F16, name="xkvt", tag="xkvt")
            XKV8 = pers.tile([P, ND, M], F8, name="xkv8", tag="xkv8")
            XQN = pers.tile([P, NNT, D], F16, name="xqn", tag="xqn")

            nc.gpsimd.dma_start(out=WQ[:, 0:2, :], in_=wq[:, 0:2, :])
            nc.gpsimd.dma_start(out=WQ[:, 2:4, :], in_=wq[:, 2:4, :])
            nc.sync.dma_start(out=XQT[:, 0:2, 0:512], in_=xqT[0, :, 0:2, :])
            nc.sync.dma_start(out=XQT[:, 2:4, 0:512], in_=xqT[0, :, 2:4, :])
            nc.scalar.dma_start(out=XQT[:, :, 512:1024], in_=xqT[1])
            nc.gpsimd.dma_start(out=WK[:], in_=wk[:])
            for c in range(nkc):
                eng = nc.sync if c % 2 == 0 else nc.scalar
                eng.dma_start(out=XKVT[:, :, _sl(c, 512)], in_=xkvT[c])
            for g in range(2):
                eng = nc.scalar if g % 2 == 0 else nc.sync
                eng.dma_start(out=XKV8[:, :, _sl(g, M // 2)], in_=xkv8[g])
            nc.gpsimd.dma_start(out=WV8[:], in_=wv8[:])
            nc.gpsimd.dma_start(out=WO8[:], in_=wo8[:])
            for h in range(2):
                nc.gpsimd.dma_start(
                    out=XQN[:, h * (NNT // 2):(h + 1) * (NNT // 2), :],
                    in_=xqn[h])
            if with_bqk:
                BQ = [pers.tile([P, 1], F32, name=f"bq{i}", tag=f"bq{i}")
                      for i in range(ND)]
                BK = [pers.tile([P, 1], F32, name=f"bk{i}", tag=f"bk{i}")
                      for i in range(ND)]
                for i in range(ND):
                    nc.gpsimd.dma_start(out=BQ[i][:],
                                        in_=bq[_sl(i)].rearrange("(a b) -> a b", b=1))
                    nc.gpsimd.dma_start(out=BK[i][:],
                                        in_=bk[_sl(i)].rearrange("(a b) -> a b", b=1))
            if with_bv:
                BV8 = pers.tile([1, D], F8, name="bv8", tag="bv8")
                ONES8 = pers.tile([1, P], F8, name="ones8", tag="ones8")
                nc.gpsimd.dma_start(out=BV8[:], in_=bv8[:])
                nc.gpsimd.memset(ONES8[:], 1.0)
            if with_bo:
                BO = pers.tile([1, D], F32, name="bo", tag="bo")
                BOB = pers.tile([P, D], F32, name="bob", tag="bob")
                nc.gpsimd.dma_start(out=BO[:], in_=bo32[:])
                nc.gpsimd.partition_broadcast(BOB[:], BO[:])

            # ---- PE warmup: ramp the tensor-engine clock while inputs load
            WARM = pers.tile([P, 640], F16, name="warm", tag="warm")
            nc.vector.memset(WARM[:], 0.5)
            with tc.tile_pool(name="wps", bufs=1, space="PSUM") as wpool:
                wps = wpool.tile([P, 512], F32, name="w", tag="w")
                NWARM = 10
                for i in range(NWARM):
                    nc.tensor.matmul(wps[:], WARM[:, 0:128], WARM[:, 128:640],
                                     start=(i == 0), stop=(i == NWARM - 1))

            # ---- projections ------------------------------------------------
            qT = pers.tile([P, ND, NQ], F16, name="qT", tag="qT")
            kT = pers.tile([P, ND, M], F16, name="kT", tag="kT")
            if KDEDUP:
                kloc = pers.tile([P, ND, M // 2], F16, name="kloc",
                                 tag="kloc")
            vv8 = pers.tile([P, NMT, D], F8, name="vv8", tag="vv8")

            with tc.tile_pool(name="pps", bufs=6, space="PSUM") as pps:
                # q projection (fp16)
                for a in range(ND):
                    pss = [pps.tile([P, 512], F32, name="projps", tag="projps")
                           for _ in range(NCH)]
                    for d in range(ND):
                        for c in range(NCH):
                            nc.tensor.matmul(pss[c][:], WQ[:, d, _sl(a)],
                                             XQT[:, d, _sl(c, 512)],
                                             start=(d == 0), stop=(d == ND - 1))
                    for c in range(NCH):
                        if with_bqk:
                            nc.vector.tensor_scalar_add(
                                qT[:, a, _sl(c, 512)], pss[c][:], BQ[a][:])
                        else:
                            nc.vector.tensor_copy(qT[:, a, _sl(c, 512)],
                                                  pss[c][:])
                # k projection over the local chunks; m-chunk outer so each
                # xkvT chunk is consumed as it lands; copies alternate
                # scalar/vector to drain 2x.
                ktgt = kloc if KDEDUP else kT
                for c in range(nkc):
                    pss = [pps.tile([P, 512], F32, name="projps", tag="projps")
                           for _ in range(ND)]
                    for a in range(ND):
                        for d in range(ND):
                            nc.tensor.matmul(pss[a][:], WK[:, d, _sl(a)],
                                             XKVT[:, d, _sl(c, 512)],
                                             start=(d == 0), stop=(d == ND - 1))
                    for a in range(ND):
                        if with_bqk:
                            if a % 2 == 0:
                                nc.scalar.add(ktgt[:, a, _sl(c, 512)],
                                              pss[a][:], add=BK[a][:])
                            else:
                                nc.vector.tensor_scalar_add(
                                    ktgt[:, a, _sl(c, 512)], pss[a][:],
                                    BK[a][:])
                        elif a % 2 == 0:
                            nc.scalar.copy(ktgt[:, a, _sl(c, 512)], pss[a][:])
                        else:
                            nc.vector.tensor_copy(ktgt[:, a, _sl(c, 512)],
                                                  pss[a][:])
                if KDEDUP:
                    # pair-exchange: kloc -> DRAM -> AllGather -> full kT.
                    # Issued on gpsimd; the v-projection below overlaps the
                    # collective latency on the PE.
                    nc.gpsimd.dma_start(out=kloc_d[0], in_=kloc[:])
                    nc.gpsimd.collective_compute(
                        "AllGather", ALU.bypass,
                        replica_groups=[[0, 1], [2, 3], [4, 5], [6, 7]],
                        ins=[kloc_d[:]], outs=[kgath_d[:]])
                    for g in range(2):
                        for half in range(2):
                            eng = nc.sync if half == 0 else nc.scalar
                            eng.dma_start(
                                out=kT[:, 2 * half:2 * half + 2,
                                       _sl(g, M // 2)],
                                in_=kgath_d[g, :, 2 * half:2 * half + 2, :])

            # ---- scores + softmax + PV + out-proj ---------------------------
            # PT16[ck][p, mj, n] = packed fp8 pair
            #   (P8[512ck+n, 256mj+2p], P8[512ck+n, 256mj+2p+1])
            PT16 = [pers.tile([P, NMJ, 512], U16, name=f"PT16{ck}",
                              tag=f"PT16{ck}") for ck in range(NCH)]
            PT8 = [PT16[ck][:].bitcast(F8).rearrange("p b (n l) -> p b l n",
                                                     l=2)
                   for ck in range(NCH)]
            OT8 = pers.tile([P, ND, NQ], F8, name="OT8", tag="OT8")
            rf = [None] * NNT  # per-tile final scale (1/(64*rowsum))

            def softmax_tile(t, spool, p8pool, stat):
                # two [P, 1024] psum half-tiles (2 banks each) so the ovy
                # pool can co-reside and PV starts right after S7.
                pt8 = p8pool.tile([P, M], F8, name="P8", tag="P8")
                halves = []
                nmh = []
                for h in range(2):
                    sps = spool.tile([P, M // 2], F32, name=f"S{h}", tag="S")
                    for a in range(ND):
                        for c in range(2):
                            mc = h * 2 + c
                            nc.tensor.matmul(sps[:, _sl(c, 512)],
                                             qT[:, a, _sl(t)],
                                             kT[:, a, _sl(mc, 512)],
                                             start=(a == 0), stop=(a == ND - 1))
                    nm = stat.tile([P, 1], F32, name=f"nm{h}", tag=f"nm{h}")
                    nc.vector.tensor_reduce(nm[:], sps[:], axis=AX.X,
                                            op=ALU.max, negate=True)
                    halves.append(sps)
                    nmh.append(nm)
                negmax = stat.tile([P, 1], F32, name="negmax", tag="negmax")
                nc.vector.tensor_tensor(negmax[:], nmh[0][:], nmh[1][:],
                                        op=ALU.min)
                # exp writes raw P8 (fp8) directly; rowsum accumulated f32.
                # Normalization is folded into the final per-tile scale.
                rsh = []
                for h in range(2):
                    rs = stat.tile([P, 1], F32, name=f"rs{h}", tag=f"rs{h}")
                    nc.scalar.activation(pt8[:, _sl(h, M // 2)], halves[h][:],
                                         ACTF.Exp, bias=negmax[:], scale=1.0,
                                         accum_out=rs[:])
                    rsh.append(rs)
                rowsum = stat.tile([P, 1], F32, name="rowsum", tag="rowsum")
                nc.vector.tensor_tensor(rowsum[:], rsh[0][:], rsh[1][:],
                                        op=ALU.add)
                recip = stat.tile([P, 1], F32, name="recip", tag="recip")
                nc.vector.reciprocal(recip[:], rowsum[:])
                rf[t] = stat.tile([P, 1], F32, name="rf", tag="rf")
                nc.vector.tensor_scalar_mul(rf[t][:], recip[:], 1.0 / 64.0)
                # packed-pair transpose, split per query-chunk tile
                ck, tt = divmod(t, 4)
                nc.sync.dma_start(out=PT16[ck][:, :, _sl(tt)],
                                  in_=pt8[:].bitcast(U16), transpose=True)

            with tc.tile_pool(name="p8pool", bufs=2) as p8pool, \
                 tc.tile_pool(name="stat", bufs=8) as stat, \
                 tc.tile_pool(name="fin", bufs=3) as fin, \
                 tc.tile_pool(name="ovy", bufs=2, space="PSUM") as ovy:
                with tc.tile_pool(name="spool", bufs=3, space="PSUM") as spool:
                    for t in range(NNT):
                        softmax_tile(t, spool, p8pool, stat)

                # v projection (fp8 DoubleRow) here: overlaps the softmax
                # pipeline drain (exp+transpose of the last tiles).
                for vt in range(NMT):
                    ps = ovy.tile([P, 512], F32, name="ot", tag="ot")
                    for g in range(2):
                        last = (g == 1) and not with_bv
                        nc.tensor.matmul(
                            ps[:], XKV8[:, 2 * g:2 * g + 2, _sl(vt)],
                            WV8[:, 2 * g:2 * g + 2, :],
                            start=(g == 0), stop=last, perf_mode=DR)
                    if with_bv:
                        nc.tensor.matmul(ps[:], ONES8[:], BV8[:],
                                         start=False, stop=True)
                    if vt % 2 == 0:
                        nc.scalar.copy(vv8[:, vt, :], ps[:])
                    else:
                        nc.vector.tensor_copy(vv8[:, vt, :], ps[:])

                with tc.tile_pool(name="ypool", bufs=2, space="PSUM") as ypool:
                    def y_tile(t):
                        ps = ypool.tile([P, D], F32, name="y", tag="y")
                        for h in range(2):
                            nc.tensor.matmul(ps[:],
                                             OT8[:, 2 * h:2 * h + 2, _sl(t)],
                                             WO8[:, 2 * h:2 * h + 2, :],
                                             start=(h == 0), stop=(h == 1),
                                             perf_mode=DR)
                        osb = fin.tile([P, D], F16, name="osb", tag="osb")
                        nc.vector.scalar_tensor_tensor(
                            out=osb[:], in0=ps[:], scalar=rf[t][:],
                            in1=XQN[:, t, :], op0=ALU.mult, op1=ALU.add)
                        if with_bo:
                            nc.vector.tensor_add(osb[:], osb[:], BOB[:])
                        nc.gpsimd.dma_start(out=out[t], in_=osb[:])

                    # PV chunk ck covers query cols 512ck..512ck+512 (tiles
                    # t=4ck..4ck+3); its Y tiles run right after so output
                    # stores spread across the PV phase.
                    for ck in range(NCH):
                        for ct in range(ND):
                            ps = ovy.tile([P, 512], F32, name="ot", tag="ot")
                            for mj in range(NMJ):
                                nc.tensor.matmul(
                                    ps[:], vv8[:, 2 * mj:2 * mj + 2, _sl(ct)],
                                    PT8[ck][:, mj, :, :],
                                    start=(mj == 0), stop=(mj == NMJ - 1),
                                    perf_mode=DR)
                            nc.scalar.activation(OT8[:, ct, _sl(ck, 512)],
                                                 ps[:], ACTF.Copy,
                                                 scale=1.0 / PSC)
                        for t in range(4 * ck, 4 * ck + 4):
                            y_tile(t)

    nc.compile()
    return nc


_BUILD_CACHE = {}


def _get_nc(with_bqk: bool, with_bv: bool, with_bo: bool):
    key = (with_bqk, with_bv, with_bo)
    if key not in _BUILD_CACHE:
        _BUILD_CACHE[key] = _build(with_bqk, with_bv, with_bo)
    return _BUILD_CACHE[key]


# key permutation for the packed-pair transpose: v'-block vt=2*mj+l holds
# keys m = 256*mj + 2*q + l (q = partition)
_J = np.arange(M)
_MIDX = 256 * (_J >> 8) + 2 * (_J & 127) + ((_J >> 7) & 1)


def kernel(query, key_value, Wq, bq, Wk, bk, Wv, bv, Wo, bo, _timing=None):
    query = np.asarray(query, dtype=np.float32)
    key_value = np.asarray(key_value, dtype=np.float32)
    Wq = np.asarray(Wq, dtype=np.float32)
    Wk = np.asarray(Wk, dtype=np.float32)
    Wv = np.asarray(Wv, dtype=np.float32)
    Wo = np.asarray(Wo, dtype=np.float32)
    bq = np.asarray(bq, dtype=np.float32)
    bk = np.asarray(bk, dtype=np.float32)
    bv = np.asarray(bv, dtype=np.float32)
    bo = np.asarray(bo, dtype=np.float32)

    with_bqk = bool(np.any(bq)) or bool(np.any(bk))
    with_bv = bool(np.any(bv))
    with_bo = bool(np.any(bo))
    nc = _get_nc(with_bqk, with_bv, with_bo)

    def packw(w, dtype, scale=1.0):
        # [512, 512] -> [128, 4, 512] with [p, d, :] = w[d*128+p, :]
        return np.ascontiguousarray(
            (w * scale).reshape(ND, P, D).transpose(1, 0, 2).astype(dtype))

    wq16 = packw(Wq, np.float16, SCALE)
    wk16 = packw(Wk, np.float16)
    wv8 = packw(Wv, NPF8, WSC)
    wo8 = packw(Wo, NPF8, WSC)
    bqs = (bq * SCALE).astype(np.float32)
    bk32 = bk.astype(np.float32)
    bv8_h = (bv * WSC).astype(NPF8).reshape(1, D)
    bo32 = bo.astype(np.float32).reshape(1, D)

    q16 = query.astype(np.float16)
    kv16 = key_value.astype(np.float16)
    kv8 = key_value.astype(NPF8)

    in_maps = []
    for core in range(N_CORES):
        b, h = divmod(core, 2)
        qh = q16[b, h * NQ:(h + 1) * NQ]                   # [1024, 512] f16
        # xqT16 [2, 128, 4, 512]: [c, p, d, nn] = qh[c*512+nn, d*128+p]
        xqT = qh.T.reshape(ND, P, NCH, 512).transpose(2, 1, 0, 3)
        # xkvT16 [4, 128, 4, 512]: [j, p, d, mm] = kv[512j+mm, d*128+p]
        xkvT = kv16[b].T.reshape(ND, P, NMC, 512).transpose(2, 1, 0, 3)
        if KDEDUP:
            xkvT = xkvT[2 * h:2 * h + 2]                   # local key half
        # xkv8 [2, 128, 4, 1024]: permuted key order
        kv8p = kv8[b][_MIDX]                                # [2048, 512] f8
        xkv8_h = kv8p.T.reshape(ND, P, 2, M // 2).transpose(2, 1, 0, 3)
        # xq16n [2, 128, 4, 512]: [h2, p, tt, j] = qh[(4*h2+tt)*128+p, j]
        xqn = qh.reshape(2, NNT // 2, P, D).transpose(0, 2, 1, 3)
        im = {
            "xqT16": np.ascontiguousarray(xqT),
            "xkvT16": np.ascontiguousarray(xkvT),
            "xkv8": np.ascontiguousarray(xkv8_h),
            "xq16n": np.ascontiguousarray(xqn),
            "wq16": wq16, "wk16": wk16, "wv8": wv8, "wo8": wo8,
            "bq": bqs, "bk": bk32, "bv8": bv8_h, "bo32": bo32,
        }
        in_maps.append(im)

    res = run_bass_kernel_spmd(nc, in_maps, list(range(N_CORES)),
                               **(_timing or {}))
    out = np.empty((B, N, D), dtype=np.float32)
    for core in range(N_CORES):
        b, h = divmod(core, 2)
        out[b, h * NQ:(h + 1) * NQ] = (
            res.results[core]["out"].reshape(NQ, D).astype(np.float32))
    if _timing is not None:
        return out, res
    return out
